# revision 42
# baseline (speedup 1.0000x reference)
"""Cross-attention (softmax over queries) on 8 Trainium2 NeuronCores.

Reference (per batch b):
    q = y @ Wq.T + bq            [N, H]
    k = x @ Wk.T + bk            [M, H]
    v = x @ Wv.T + bv            [M, D]
    dots = (q @ k.T) * H**-0.5   [N, M]
    attn = softmax(dots, axis=0) (over queries n, per key column m)
    out  = attn @ v              [N, D]

Sharding: data-parallel over batch B=8, one batch per core (SPMD).

Device algorithm (per core, matmuls fp16 with fp32 PSUM accumulation):
  Host pre-transposes/casts everything into exact SBUF layout (partition
  dim first, contiguous per partition) so each DMA is one fat segment;
  the q-side weight folds the 1/sqrt(H) scale. DMAs issue on one queue
  in strict first-use order so startup-critical transfers get the
  engines first.
  A. project qT[h,n], kT[h,m] straight from the host-transposed inputs
     (q/k biases added by the ACT psum->sbuf copy, per-partition).
  C. per 128-row key chunk mc: V-projection chunk into PSUM, dotsT[m,n]
     into two [128,1024] PSUM banks, column max (DVE), fused exp+rowsum
     on ACT into attnT e4m3; then v' = (pv + bv)(1/colsum) via DVE add
     of a pre-broadcast bias + ACT scale-copy, split v' = v_hi + v_lo
     (both e4m3, v_lo at natural scale, near-fp16 combined accuracy).
  D. out[n,d] = sum_m attnT[m,n] * v'[m,d]; fp8 DoubleRow matmuls
     (K=256/instr, 2x fp16 FLOPs), hi+lo passes accumulate into one
     PSUM group; PSUM tiles alternate between the pp and dots pools so
     6 chains stay in flight across the C->D transition.
"""

from contextlib import ExitStack

import numpy as np

import concourse.mybir as mybir
import concourse.tile as tile
from concourse import bacc
from concourse.bass_utils import run_bass_kernel_spmd

F32 = mybir.dt.float32
F16 = mybir.dt.float16
F8 = mybir.dt.float8e4
Exp = mybir.ActivationFunctionType.Exp
DR = mybir.MatmulPerfMode.DoubleRow
AX = mybir.AxisListType.X

B, N, M, C, H, D = 8, 2048, 2048, 1024, 512, 1024
P = 128
NT, MT, CCH, HC = N // P, M // P, C // P, H // P  # 16, 16, 8, 4
SCALE = (C // 2) ** -0.5

_CACHE = {}


def _build_nc():
    nc = bacc.Bacc("TRN2", target_bir_lowering=False, debug=False)

    # All inputs arrive in SBUF layout (partition dim first, fully
    # contiguous per partition) so every DMA is fat-segment and cheap.
    yt_d = nc.dram_tensor("yt", [P, 4, CCH, 512], F16, kind="ExternalInput").ap()
    xt_d = nc.dram_tensor("xt", [P, 4, CCH, 512], F16, kind="ExternalInput").ap()
    wqt_d = nc.dram_tensor("wqt", [P, CCH, H], F16, kind="ExternalInput").ap()
    wkt_d = nc.dram_tensor("wkt", [P, CCH, H], F16, kind="ExternalInput").ap()
    wvt_d = nc.dram_tensor("wvt", [P, CCH, D], F16, kind="ExternalInput").ap()
    bq_d = nc.dram_tensor("bq", [P, HC], F32, kind="ExternalInput").ap()
    bk_d = nc.dram_tensor("bk", [P, HC], F32, kind="ExternalInput").ap()
    bv_d = nc.dram_tensor("bv", [D], F16, kind="ExternalInput").ap()
    out_d = nc.dram_tensor("out", [N, D], F32, kind="ExternalOutput").ap()

    out_r = out_d.rearrange("(t p) d -> p t d", p=P)

    with tile.TileContext(nc) as tc:
        with (
            tc.tile_pool(name="persist", bufs=1) as pers,
            tc.tile_pool(name="stats", bufs=1) as stats,
            tc.tile_pool(name="xT_pool", bufs=1) as xTp,
            tc.tile_pool(name="ps_pp", bufs=4, space="PSUM") as psPP,
            tc.tile_pool(name="ps_c", bufs=1, space="PSUM") as psC,
        ):
            qT = pers.tile([P, HC, N], F16, tag="qT")  # [h%128, h//128, n] 2MB
            kT = pers.tile([P, HC, M], F16, tag="kT")  # 2MB
            # v' = (x@WvT + bv) / colsum, split v' = v_hi + v_lo (both e4m3,
            # v_lo at natural scale) so fp8 DoubleRow matmuls accumulate both
            # halves into one PSUM group at near-fp16 accuracy.
            v_hi = pers.tile([P, MT, D], F8, tag="v_hi")  # [m%128, m//128, d]
            v_lo = pers.tile([P, MT, D], F8, tag="v_lo")
            ones = pers.tile([1, P], F16, tag="ones")
            nc.vector.memset(ones[:], 1.0)
            bvb = pers.tile([P, D], F16, tag="bvb")  # bv bcast along partitions

            sums = stats.tile([P, MT], F32, tag="sums")
            rsum = stats.tile([P, MT], F32, tag="rsum")
            bq_sb = stats.tile([P, HC], F32, tag="bq")  # [h%128, h//128]
            bk_sb = stats.tile([P, HC], F32, tag="bk")
            bv_sb = stats.tile([1, D], F16, tag="bv")

            xT = xTp.tile([P, 4, CCH, 512], F16, tag="xT")  # [p, mj, cc, ms]
            wv_sb = pers.tile([P, CCH, D], F16, tag="wv")  # 2MB

            # ---------- Phase A: q/k projections off host-transposed inputs ----
            with (
                tc.tile_pool(name="yT_pool", bufs=1) as yTp,
                tc.tile_pool(name="w_pool", bufs=1) as wp,
            ):
                wq_sb = wp.tile([P, CCH, H], F16, tag="wq")  # [c%128, c//128, h]
                wk_sb = wp.tile([P, CCH, H], F16, tag="wk")
                yT = yTp.tile([P, 4, CCH, 512], F16, tag="yT")

                # PE warmup: dummy matmuls with no DMA dependencies run during
                # the initial input-transfer wait so the PE clock is fully
                # ramped (pstate) when the first projection lands
                warm = wp.tile([P, 512], F16, tag="warm")
                nc.vector.memset(warm[:], 0.0)
                for _ in range(3):
                    pw = psPP.tile([P, 512], F32, tag="pp")
                    for w in range(4):
                        nc.tensor.matmul(
                            pw[:],
                            warm[:, 0:P],
                            warm[:],
                            start=(w == 0),
                            stop=(w == 3),
                        )

                # All input DMAs on the sync queue in strict first-use order:
                # DMA engines are shared, so concurrently-issued later-needed
                # transfers (x, wv) would steal bandwidth from the startup-
                # critical y0+wq and delay the first matmul.
                # lowest-cc slabs of BOTH y0 and wq issue first (one per
                # queue) since the first projection chain consumes cc in order
                # four issue queues (each feeds its own DMA ring set) so
                # consecutive cc-pairs of the first block arrive in parallel
                nc.sync.dma_start(yT[:, 0, 0:2], yt_d[:, 0, 0:2])
                nc.gpsimd.dma_start(wq_sb[:, 0:2], wqt_d[:, 0:2])
                nc.scalar.dma_start(yT[:, 0, 2:4], yt_d[:, 0, 2:4])
                nc.sync.dma_start(wq_sb[:, 2:4], wqt_d[:, 2:4])
                nc.gpsimd.dma_start(yT[:, 0, 4:6], yt_d[:, 0, 4:6])
                nc.scalar.dma_start(wq_sb[:, 4:6], wqt_d[:, 4:6])
                nc.sync.dma_start(yT[:, 0, 6:8], yt_d[:, 0, 6:8])
                nc.gpsimd.dma_start(wq_sb[:, 6:8], wqt_d[:, 6:8])
                nc.sync.dma_start(yT[:, 1], yt_d[:, 1])
                nc.sync.dma_start(bq_sb[:], bq_d)
                nc.sync.dma_start(xT[:, 0], xt_d[:, 0])
                nc.sync.dma_start(yT[:, 2], yt_d[:, 2])
                nc.sync.dma_start(yT[:, 3], yt_d[:, 3])
                nc.sync.dma_start(wk_sb[:], wkt_d)
                nc.sync.dma_start(bk_sb[:], bk_d)
                nc.sync.dma_start(xT[:, 1], xt_d[:, 1])
                nc.sync.dma_start(bv_sb[:], bv_d[None, :])
                nc.sync.dma_start(wv_sb[:, :, 0:512], wvt_d[:, :, 0:512])
                nc.sync.dma_start(wv_sb[:, :, 512:D], wvt_d[:, :, 512:D])
                nc.sync.dma_start(xT[:, 2], xt_d[:, 2])
                nc.sync.dma_start(xT[:, 3], xt_d[:, 3])

                def project_j(dst, w_sb, b_sb, src_T, j):
                    # one 512-wide column block of a projection, all hc chunks
                    for hc in range(HC):
                        pp = psPP.tile([P, 512], F32, tag="pp")
                        for cc in range(CCH):
                            nc.tensor.matmul(
                                pp[:],
                                w_sb[:, cc, hc * P : (hc + 1) * P],
                                src_T[:, j, cc, :],
                                start=(cc == 0),
                                stop=(cc == CCH - 1),
                            )
                        # ACT copy: psum -> f16, + per-partition bias
                        nc.scalar.add(
                            dst[:, hc, j * 512 : (j + 1) * 512],
                            pp[:],
                            b_sb[:, hc : hc + 1],
                        )

                def project_j0_pipelined(dst, w_sb, b_sb, src_T):
                    # first j-block is DMA-arrival-bound: open all 4 hc PSUM
                    # chains at once and accumulate in cc-pair passes, so each
                    # pass consumes exactly the y/wq slab pair that has
                    # already landed instead of stalling mid-chain
                    pps = []
                    for _hc in range(HC):
                        ppx = psPP.tile([P, 512], F32, tag="pp")
                        pps.append(ppx)
                    for cp in range(4):
                        for hc in range(HC):
                            for cc in (2 * cp, 2 * cp + 1):
                                nc.tensor.matmul(
                                    pps[hc][:],
                                    w_sb[:, cc, hc * P : (hc + 1) * P],
                                    src_T[:, 0, cc, :],
                                    start=(cc == 0),
                                    stop=(cc == CCH - 1),
                                )
                    for hc in range(HC):
                        nc.scalar.add(dst[:, hc, 0:512], pps[hc][:], b_sb[:, hc : hc + 1])

                with nc.named_scope("A_yq"):
                    project_j0_pipelined(qT, wq_sb, bq_sb, yT)
                    for j in range(1, 4):
                        project_j(qT, wq_sb, bq_sb, yT, j)
                with nc.named_scope("A_xk"):
                    for j in range(4):
                        project_j(kT, wk_sb, bk_sb, xT, j)

            # ---------- Phase C: V-proj chunks interleaved with dots/softmax ----
            with (
                tc.tile_pool(name="late", bufs=1) as late,
                tc.tile_pool(name="sc", bufs=4) as sc,
            ):
                attnT = late.tile([P, MT, N], F8, tag="attnT")  # 4MB

                # one-time: bvb[p, d] = bv[d] on every partition (K=1 matmul
                # broadcast through PSUM, then ACT copy to SBUF f16); placed
                # after phase A so it doesn't block the in-order PE queue at
                # startup waiting for the late bv DMA
                for dh in range(2):
                    pb = psPP.tile([P, 512], F32, tag="pp")
                    nc.tensor.matmul(
                        pb[:],
                        ones[:],
                        bv_sb[:, dh * 512 : (dh + 1) * 512],
                        start=True,
                        stop=True,
                    )
                    nc.scalar.copy(bvb[:, dh * 512 : (dh + 1) * 512], pb[:])

                def v_chunk(mc):
                    # v[m, d] for m-chunk mc: lhsT = xT (c,m), rhs = wv (c,d).
                    # The bias add drains the PSUM result into an f16 SBUF
                    # staging tile right away: pv is freed ~6us earlier than
                    # if the extraction read it, killing the PSUM WAR stall
                    # that throttled the next chunk's V matmuls; hi/lo
                    # extraction reads the staging tile once rsum[mc] lands.
                    vss = []
                    for dh in range(2):
                        pv = psPP.tile([P, 512], F32, tag="pp")
                        for cc in range(CCH):
                            nc.tensor.matmul(
                                pv[:],
                                xT[:, mc // 4, cc, (mc % 4) * P : (mc % 4 + 1) * P],
                                wv_sb[:, cc, dh * 512 : (dh + 1) * 512],
                                start=(cc == 0),
                                stop=(cc == CCH - 1),
                            )
                        vs = sc.tile([P, 512], F16, tag="vs")
                        nc.vector.tensor_tensor(
                            vs[:],
                            pv[:],
                            bvb[:, dh * 512 : (dh + 1) * 512],
                            mybir.AluOpType.add,
                        )
                        vss.append(vs)
                    return vss

                def dots_chunk(mc, vss):
                    # 4 single-bank dots tiles (same 4 banks as 2x[P,1024])
                    # so the rotation granularity matches the max->exp chain:
                    # dots(mc+1) only waits for exp(mc, j=0), which lands
                    # within the next v_chunk's PE cover
                    pds = []
                    for j in range(4):
                        pd = psC.tile([P, 512], F32, tag=f"dots{j}")
                        for hc in range(HC):
                            nc.tensor.matmul(
                                pd[:],
                                kT[:, hc, mc * P : (mc + 1) * P],
                                qT[:, hc, j * 512 : (j + 1) * 512],
                                start=(hc == 0),
                                stop=(hc == HC - 1),
                            )
                        pds.append(pd)
                    pmax = sc.tile([P, 4], F32, tag="pmax")
                    for j in range(4):
                        nc.vector.reduce_max(pmax[:, j : j + 1], pds[j][:], axis=AX)
                    negmax = sc.tile([P, 1], F32, tag="negmax")
                    nc.vector.reduce_max(negmax[:], pmax[:], axis=AX, negate=True)
                    ssum = sc.tile([P, 4], F32, tag="ssum")
                    for j in range(4):
                        nc.scalar.activation(
                            out=attnT[:, mc, j * 512 : (j + 1) * 512],
                            in_=pds[j][:],
                            func=Exp,
                            bias=negmax[:],
                            accum_out=ssum[:, j : j + 1],
                        )
                    nc.vector.tensor_reduce(
                        sums[:, mc : mc + 1],
                        ssum[:],
                        axis=AX,
                        op=mybir.AluOpType.add,
                    )
                    nc.vector.reciprocal(rsum[:, mc : mc + 1], sums[:, mc : mc + 1])
                    # v' = (pv + bv) * (1/colsum); v_hi = e4m3(v'),
                    # v_lo = e4m3(v' - v_hi)  (bias already added in v_chunk)
                    for dh, vs in enumerate(vss):
                        dsl = slice(dh * 512, (dh + 1) * 512)
                        nc.scalar.mul(v_hi[:, mc, dsl], vs[:], rsum[:, mc : mc + 1])
                        nc.vector.scalar_tensor_tensor(
                            out=v_lo[:, mc, dsl],
                            in0=vs[:],
                            scalar=rsum[:, mc : mc + 1],
                            in1=v_hi[:, mc, dsl],
                            op0=mybir.AluOpType.mult,
                            op1=mybir.AluOpType.subtract,
                        )

                with nc.named_scope("C_loop"):
                    for mc in range(MT):
                        vss = v_chunk(mc)
                        dots_chunk(mc, vss)

                # ---------- Phase D: out = attnT^T @ v' ----------
                with (
                    tc.tile_pool(name="so", bufs=4) as so,
                    nc.named_scope("D_out"),
                ):
                    for ntc in range(NT):
                        for dh in range(2):
                            # alternate PSUM pools: C's final pv tiles hold 2
                            # of psPP's 4 buffers until the last extraction,
                            # which would leave only 2 chains in flight at
                            # the C->D transition; borrowing the (idle) dots
                            # buffers keeps 6 going so the PE never stalls
                            idx = 2 * ntc + dh
                            if idx % 2 == 0:
                                po = psPP.tile([P, 512], F32, tag="pp")
                            else:
                                po = psC.tile(
                                    [P, 512], F32, tag=f"dots{(idx // 2) % 4}"
                                )
                            dsl = slice(dh * 512, (dh + 1) * 512)
                            # fp8 DoubleRow: each matmul consumes 2 m-chunks
                            # (K=256); hi and lo passes share one PSUM group
                            for vi, vsrc in enumerate((v_hi, v_lo)):
                                for mp in range(MT // 2):
                                    nc.tensor.matmul(
                                        po[:],
                                        attnT[
                                            :,
                                            2 * mp : 2 * mp + 2,
                                            ntc * P : (ntc + 1) * P,
                                        ],
                                        vsrc[:, 2 * mp : 2 * mp + 2, dsl],
                                        start=(vi == 0 and mp == 0),
                                        stop=(vi == 1 and mp == MT // 2 - 1),
                                        perf_mode=DR,
                                    )
                            ot = so.tile([P, 512], F32, tag="ot")
                            nc.scalar.copy(ot[:], po[:])
                            nc.sync.dma_start(
                                out_r[:, ntc, dh * 512 : (dh + 1) * 512], ot[:]
                            )

    nc.finalize()
    return nc


def _get_nc():
    if "nc" not in _CACHE:
        _CACHE["nc"] = _build_nc()
    return _CACHE["nc"]


def _prep_in_maps(y, x, Wq, bq, Wk, bk, Wv, bv):
    y = np.asarray(y, dtype=np.float32)
    x = np.asarray(x, dtype=np.float32)

    def act_layout(a):
        # [B, n, c] f32 -> [B, p, j, o, ns] f16 with c = o*128+p, n = j*512+ns
        a16 = np.transpose(a, (0, 2, 1)).astype(np.float16)  # [B, c, n]
        a16 = a16.reshape(B, CCH, P, 4, 512)
        return np.ascontiguousarray(np.transpose(a16, (0, 2, 3, 1, 4)))

    def w_layout(w):
        # [out, in=c] -> wT [c, out] -> [p, o, out] with c = o*128+p
        wt = np.asarray(w).T.astype(np.float16)
        return np.ascontiguousarray(
            wt.reshape(CCH, P, wt.shape[1]).transpose(1, 0, 2)
        )

    def b_layout(b):
        # [h] -> [p, o] with h = o*128+p
        b32 = np.asarray(b, dtype=np.float32)
        return np.ascontiguousarray(b32.reshape(-1, P).T)

    yt = act_layout(y)
    xt = act_layout(x)
    wqt = w_layout(np.asarray(Wq) * SCALE)
    wkt = w_layout(Wk)
    wvt = w_layout(Wv)
    bq32 = b_layout(np.asarray(bq) * SCALE)
    bk32 = b_layout(bk)
    bv16 = np.asarray(bv).astype(np.float16)
    return [
        {
            "yt": yt[b],
            "xt": xt[b],
            "wqt": wqt,
            "wkt": wkt,
            "wvt": wvt,
            "bq": bq32,
            "bk": bk32,
            "bv": bv16,
        }
        for b in range(B)
    ]


def run(inputs, trace=False, trace_cores=None):
    nc = _get_nc()
    in_maps = _prep_in_maps(**inputs)
    r = run_bass_kernel_spmd(
        nc, in_maps, list(range(B)), trace=trace, trace_cores=trace_cores
    )
    out = np.stack([r.results[b]["out"] for b in range(B)], axis=0)
    return out, r


def kernel(**inputs) -> np.ndarray:
    out, _ = run(inputs, trace=False)
    return out
